# revision 5
# baseline (speedup 1.0000x reference)
"""Trainium2 Bass kernel for nn_BiologicalNormalization.

Math: three chained per-sample LayerNorms (affine params gathered per-sample
by id on the host). The trailing gated blend ``x*sigmoid(xW+b) +
x*(1-sigmoid(xW+b))`` is mathematically the identity, so the kernel returns
the triple-LayerNorm result directly.

Distribution: pure data parallelism — batch 2048 is split into 8 shards of
256 samples, one per NeuronCore. Per-id affine tables are gathered to
per-sample rows on the host (tiny), so each core only sees dense tensors.

Per-core layout: partition dim = 128 samples (2 groups of 128), free dim =
D=512; sequence positions processed in chunks of K for 1 MiB DMAs.
"""

import numpy as np

import concourse.bacc as bacc
import concourse.mybir as mybir
from concourse.tile import TileContext
from concourse.bass_utils import run_bass_kernel_spmd

NCORES = 8
B, S, D = 2048, 128, 512
BS = B // NCORES  # samples per core
P = 128  # SBUF partitions (samples per group)
NGRP = BS // P
K = 4  # sequence positions per DMA chunk
EPS = 1e-5
FP = mybir.dt.float32
PARAM_NAMES = ("g1", "b1", "g2", "b2", "g3", "b3")


def _layernorm_chain(nc, ps, pi, src, params, dst, eps_tile):
    """src [P, D] -> dst [P, D]: LN(LN(LN(src,g1,b1),g2,b2),g3,b3)."""
    cur = src
    for i in range(3):
        g = params[PARAM_NAMES[2 * i]]
        b = params[PARAM_NAMES[2 * i + 1]]
        stats = ps.tile([P, 6], FP, tag="stats")
        nc.vector.bn_stats(out=stats, in_=cur)
        mv = ps.tile([P, 2], FP, tag="mv")
        nc.vector.bn_aggr(out=mv, in_=stats)
        mean = mv[:, 0:1]
        var = mv[:, 1:2]
        std = ps.tile([P, 1], FP, tag="std")
        nc.scalar.activation(
            out=std, in_=var, func=mybir.ActivationFunctionType.Sqrt, bias=eps_tile
        )
        rstd = ps.tile([P, 1], FP, tag="rstd")
        nc.vector.reciprocal(out=rstd, in_=std)
        t = pi.tile([P, D], FP, tag="t")
        nc.vector.scalar_tensor_tensor(
            out=t,
            in0=cur,
            scalar=mean,
            in1=g,
            op0=mybir.AluOpType.subtract,
            op1=mybir.AluOpType.mult,
        )
        y = dst if i == 2 else pi.tile([P, D], FP, tag=f"y{i}")
        nc.vector.scalar_tensor_tensor(
            out=y,
            in0=t,
            scalar=rstd,
            in1=b,
            op0=mybir.AluOpType.mult,
            op1=mybir.AluOpType.add,
        )
        cur = y


def _build():
    nc = bacc.Bacc("TRN2", target_bir_lowering=False, debug=False, num_devices=NCORES)
    x = nc.declare_dram_parameter("x", [BS, S, D], FP, isOutput=False).ap()
    prm = {
        k: nc.declare_dram_parameter(k, [BS, D], FP, isOutput=False).ap()
        for k in PARAM_NAMES
    }
    out = nc.declare_dram_parameter("out", [BS, S, D], FP, isOutput=True).ap()

    with TileContext(nc) as tc:
        with (
            tc.tile_pool(name="params", bufs=2) as pp,
            tc.tile_pool(name="xin", bufs=3) as px,
            tc.tile_pool(name="yout", bufs=3) as po,
            tc.tile_pool(name="inter", bufs=3) as pi,
            tc.tile_pool(name="small", bufs=8) as ps,
            tc.tile_pool(name="singles", bufs=1) as pc,
        ):
            eps_tile = pc.tile([P, 1], FP)
            nc.vector.memset(eps_tile, EPS)
            for grp in range(NGRP):
                b0 = grp * P
                pt = {}
                for kname in PARAM_NAMES:
                    t = pp.tile([P, D], FP, tag=kname)
                    nc.sync.dma_start(out=t, in_=prm[kname][b0 : b0 + P, :])
                    pt[kname] = t
                for c in range(S // K):
                    s0 = c * K
                    xt = px.tile([P, K, D], FP)
                    nc.sync.dma_start(out=xt, in_=x[b0 : b0 + P, s0 : s0 + K, :])
                    ot = po.tile([P, K, D], FP)
                    for j in range(K):
                        _layernorm_chain(
                            nc, ps, pi, xt[:, j, :], pt, ot[:, j, :], eps_tile
                        )
                    nc.sync.dma_start(out=out[b0 : b0 + P, s0 : s0 + K, :], in_=ot)
    nc.compile()
    return nc


_NC = None


def _get_nc():
    global _NC
    if _NC is None:
        _NC = _build()
    return _NC


def kernel(
    x,
    pathway_ids,
    compartment_ids,
    cell_type_ids,
    pathway_gamma,
    pathway_beta,
    compartment_gamma,
    compartment_beta,
    cell_type_gamma,
    cell_type_beta,
    W=None,
    b=None,
    **_unused,
):
    x = np.ascontiguousarray(np.asarray(x, dtype=np.float32))
    pid = np.asarray(pathway_ids).astype(np.int64)
    cid = np.asarray(compartment_ids).astype(np.int64)
    tid = np.asarray(cell_type_ids).astype(np.int64)
    # Host-side gather of per-sample affine rows (tiny: 6 x [B, D] f32).
    full = {
        "g1": np.asarray(pathway_gamma, np.float32)[pid],
        "b1": np.asarray(pathway_beta, np.float32)[pid],
        "g2": np.asarray(compartment_gamma, np.float32)[cid],
        "b2": np.asarray(compartment_beta, np.float32)[cid],
        "g3": np.asarray(cell_type_gamma, np.float32)[tid],
        "b3": np.asarray(cell_type_beta, np.float32)[tid],
    }
    in_maps = []
    for i in range(NCORES):
        sl = slice(i * BS, (i + 1) * BS)
        m = {"x": x[sl]}
        for k, v in full.items():
            m[k] = np.ascontiguousarray(v[sl])
        in_maps.append(m)

    nc = _get_nc()
    res = run_bass_kernel_spmd(nc, in_maps, core_ids=list(range(NCORES)))
    return np.concatenate([res.results[i]["out"] for i in range(NCORES)], axis=0)


# revision 9
# speedup vs baseline: 260.1370x; 260.1370x over previous
"""Trainium2 Bass kernel for nn_BiologicalNormalization.

Math: three chained per-sample LayerNorms (affine params gathered per-sample
by id on the host). The trailing gated blend ``x*sigmoid(xW+b) +
x*(1-sigmoid(xW+b))`` is mathematically the identity, so the kernel returns
the triple-LayerNorm result directly.

Distribution: pure data parallelism — batch 2048 is split into 8 shards of
256 samples, one per NeuronCore. Per-id affine tables are gathered to
per-sample rows on the host (tiny), so each core only sees dense tensors.

Per-core layout: partition dim = 128 samples (2 groups of 128), free dim =
D=512; sequence positions processed in chunks of K for 1 MiB DMAs.
"""

import numpy as np

import concourse.bacc as bacc
import concourse.mybir as mybir
from concourse.tile import TileContext
from concourse.bass_utils import run_bass_kernel_spmd

NCORES = 8
B, S, D = 2048, 128, 512
BS = B // NCORES  # samples per core
P = 128  # SBUF partitions (samples per group)
NGRP = BS // P
K = 4  # sequence positions per DMA chunk
EPS = 1e-5
FP = mybir.dt.float32
PARAM_NAMES = ("g1", "b1", "g2", "b2", "g3", "b3")


def _layernorm_chain(nc, ps, pi, src, params, dst, eps_tile):
    """src [P, D] -> dst [P, D]: LN(LN(LN(src,g1,b1),g2,b2),g3,b3)."""
    cur = src
    for i in range(3):
        g = params[PARAM_NAMES[2 * i]]
        b = params[PARAM_NAMES[2 * i + 1]]
        stats = ps.tile([P, 6], FP, tag="stats")
        nc.vector.bn_stats(out=stats, in_=cur)
        mv = ps.tile([P, 2], FP, tag="mv")
        nc.vector.bn_aggr(out=mv, in_=stats)
        mean = mv[:, 0:1]
        var = mv[:, 1:2]
        std = ps.tile([P, 1], FP, tag="std")
        nc.scalar.activation(
            out=std, in_=var, func=mybir.ActivationFunctionType.Sqrt, bias=eps_tile
        )
        rstd = ps.tile([P, 1], FP, tag="rstd")
        nc.vector.reciprocal(out=rstd, in_=std)
        t = pi.tile([P, D], FP, tag="t")
        nc.vector.scalar_tensor_tensor(
            out=t,
            in0=cur,
            scalar=mean,
            in1=g,
            op0=mybir.AluOpType.subtract,
            op1=mybir.AluOpType.mult,
        )
        y = dst if i == 2 else pi.tile([P, D], FP, tag=f"y{i}")
        nc.vector.scalar_tensor_tensor(
            out=y,
            in0=t,
            scalar=rstd,
            in1=b,
            op0=mybir.AluOpType.mult,
            op1=mybir.AluOpType.add,
        )
        cur = y


def _build():
    nc = bacc.Bacc("TRN2", target_bir_lowering=False, debug=False, num_devices=NCORES)
    x = nc.declare_dram_parameter("x", [BS, S, D], FP, isOutput=False).ap()
    prm = {
        k: nc.declare_dram_parameter(k, [BS, D], FP, isOutput=False).ap()
        for k in PARAM_NAMES
    }
    out = nc.declare_dram_parameter("out", [BS, S, D], FP, isOutput=True).ap()

    with TileContext(nc) as tc:
        with (
            tc.tile_pool(name="params", bufs=2) as pp,
            tc.tile_pool(name="xin", bufs=3) as px,
            tc.tile_pool(name="yout", bufs=3) as po,
            tc.tile_pool(name="inter", bufs=3) as pi,
            tc.tile_pool(name="small", bufs=8) as ps,
            tc.tile_pool(name="singles", bufs=1) as pc,
        ):
            eps_tile = pc.tile([P, 1], FP)
            nc.vector.memset(eps_tile, EPS)
            for grp in range(NGRP):
                b0 = grp * P
                pt = {}
                for kname in PARAM_NAMES:
                    t = pp.tile([P, D], FP, tag=kname)
                    nc.sync.dma_start(out=t, in_=prm[kname][b0 : b0 + P, :])
                    pt[kname] = t
                for c in range(S // K):
                    s0 = c * K
                    xt = px.tile([P, K, D], FP)
                    nc.sync.dma_start(out=xt, in_=x[b0 : b0 + P, s0 : s0 + K, :])
                    ot = po.tile([P, K, D], FP)
                    for j in range(K):
                        _layernorm_chain(
                            nc, ps, pi, xt[:, j, :], pt, ot[:, j, :], eps_tile
                        )
                    nc.sync.dma_start(out=out[b0 : b0 + P, s0 : s0 + K, :], in_=ot)
    nc.compile()
    return nc


class _Runner:
    """Persistent compiled SPMD executor for the Bass graph.

    Mirrors bass2jax.run_bass_via_pjrt but keeps the jitted callable and the
    device mesh alive so repeated calls don't retrace/recompile.
    """

    def __init__(self, nc):
        import jax
        import concourse.bass2jax as bass2jax
        from jax.experimental.shard_map import shard_map
        from jax.sharding import Mesh, NamedSharding, PartitionSpec

        bass2jax.install_neuronx_cc_hook()
        self._jax = jax
        self._nc = nc

        partition_name = (
            nc.partition_id_tensor.name if nc.partition_id_tensor else None
        )
        in_names = []
        out_names = []
        out_avals = []
        for alloc in nc.m.functions[0].allocations:
            if not isinstance(alloc, mybir.MemoryLocationSet):
                continue
            name = alloc.memorylocations[0].name
            if alloc.kind == "ExternalInput":
                if name != partition_name:
                    in_names.append(name)
            elif alloc.kind == "ExternalOutput":
                out_names.append(name)
                out_avals.append(
                    jax.core.ShapedArray(
                        tuple(alloc.tensor_shape), mybir.dt.np(alloc.dtype)
                    )
                )
        self.in_names = list(in_names)
        self.out_names = out_names
        self.out_avals = out_avals
        n_params = len(in_names)
        all_in_names = in_names + out_names
        if partition_name is not None:
            all_in_names = all_in_names + [partition_name]

        def _body(*args):
            operands = list(args)
            if partition_name is not None:
                operands.append(bass2jax.partition_id_tensor())
            outs = bass2jax._bass_exec_p.bind(
                *operands,
                out_avals=tuple(out_avals),
                in_names=tuple(all_in_names),
                out_names=tuple(out_names),
                lowering_input_output_aliases=(),
                sim_require_finite=True,
                sim_require_nnan=True,
                nc=nc,
            )
            return tuple(outs)

        devices = jax.devices()[:NCORES]
        self.mesh = Mesh(np.asarray(devices), ("core",))
        self.sharding = NamedSharding(self.mesh, PartitionSpec("core"))
        n_outs = len(out_names)
        donate = tuple(range(n_params, n_params + n_outs))
        self._exec = jax.jit(
            shard_map(
                _body,
                mesh=self.mesh,
                in_specs=(PartitionSpec("core"),) * (n_params + n_outs),
                out_specs=(PartitionSpec("core"),) * n_outs,
                check_rep=False,
            ),
            donate_argnums=donate,
            keep_unused=True,
        )

        def _mk_zeros():
            import jax.numpy as jnp

            return tuple(
                jnp.zeros((NCORES * a.shape[0], *a.shape[1:]), a.dtype)
                for a in out_avals
            )

        self._zeros = jax.jit(
            _mk_zeros, out_shardings=(self.sharding,) * n_outs
        )

    def put_inputs(self, concat_ins):
        """Transfer concatenated (axis0 = NCORES*shard) inputs to devices."""
        return [
            self._jax.device_put(v, self.sharding) for v in concat_ins
        ]

    def run(self, dev_ins):
        """One execution; returns tuple of global output arrays (device)."""
        zeros = self._zeros()
        return self._exec(*dev_ins, *zeros)


_RUNNER = None


def get_runner():
    global _RUNNER
    if _RUNNER is None:
        _RUNNER = _Runner(_build())
    return _RUNNER


def kernel(
    x,
    pathway_ids,
    compartment_ids,
    cell_type_ids,
    pathway_gamma,
    pathway_beta,
    compartment_gamma,
    compartment_beta,
    cell_type_gamma,
    cell_type_beta,
    W=None,
    b=None,
    **_unused,
):
    x = np.ascontiguousarray(np.asarray(x, dtype=np.float32))
    pid = np.asarray(pathway_ids).astype(np.int64)
    cid = np.asarray(compartment_ids).astype(np.int64)
    tid = np.asarray(cell_type_ids).astype(np.int64)
    # Host-side gather of per-sample affine rows (tiny: 6 x [B, D] f32).
    full = {
        "g1": np.asarray(pathway_gamma, np.float32)[pid],
        "b1": np.asarray(pathway_beta, np.float32)[pid],
        "g2": np.asarray(compartment_gamma, np.float32)[cid],
        "b2": np.asarray(compartment_beta, np.float32)[cid],
        "g3": np.asarray(cell_type_gamma, np.float32)[tid],
        "b3": np.asarray(cell_type_beta, np.float32)[tid],
    }
    runner = get_runner()
    by_name = {"x": x, **full}
    concat_ins = [by_name[name] for name in runner.in_names]
    dev_ins = runner.put_inputs(concat_ins)
    outs = runner.run(dev_ins)
    return np.asarray(outs[0])
